# revision 11
# baseline (speedup 1.0000x reference)
"""MDCT kernel for Trainium2 (8 NeuronCores, batch-parallel), folded DCT-IV form.

Math: frame f (hop N=1024, len 2N, center-padded) folds to an N-vector u and
out[f] = DCT-IV(u).  With x2 = x.reshape(1024, 1024) and y1 = w[:N]*x2[r],
y2 = w[N:]*x2[r] (per-row windowing):
    u[f, m]      = -y2[f, 511-m] - y2[f, 512+m]      (m < 512,  row f)
    u[f, 512+p]  =  y1[f-1, p]   - y1[f-1, 1023-p]   (p < 512,  row f-1)
so each x2 row r yields uLo[r] (frame r) and uHi[r] (frame r+1), and
    out[f, k] = sum_m u[f, m] * D4[m, k],   D4 = sqrt(2/N) DCT-IV matrix.

This halves the matmul contraction (1024 vs 2048) vs the direct form.  The
fold runs on the vector engine (reversals are negative-stride APs), u is
transposed on the PE in bf16 (1 cyc/row), and the DCT matmuls run in bf16.

Only the LEFT half of D4 is DMA'd (1.05 MB instead of 2.1 MB): the DCT-IV
matrix satisfies  D4[m, 512+k] = s_m*sqrt(2)*D4[m, k] - D4[m, 511-k]  with
s_m = +1 for m%4 in {0,3} else -1.  The right half is derived ON THE PE
(idle during the fill) as a 2-matmul PSUM chain per 128-row chunk:
diag(s*sqrt2) @ Dl  accumulated with  (-I) @ Dl-reversed-columns, then an
ACT copy back to SBUF.  (GPSIMD tensor_scalar takes 7.6us per [128,512]
chunk - far too slow; DVE has no spare capacity early.)

Schedule notes (v5, from NTFF profiles of baseline=53.8us, v2=66.8us,
v3=124us, v4=57.6us):
- the NEFF preamble (engine iram loads, const memsets, a global barrier)
  ends ~7.2 us; nothing (including DMA) starts before that.
- concurrent DMA streams share ~400 GB/s; completion of any transfer is
  set by cumulative bytes ahead of it, so the DCT gate is the byte count
  of w + x0 + Dl (~1.8 MB) rather than instruction order tricks.
- the HAM clock gate resets on any PE idle gap: warmup transposes must
  keep the PE continuously busy from the preamble barrier until real
  work arrives, or the DCT stream starts at 1.2 GHz (v4 lost ~2 us).
- per-tile chains: pa streams Dl with chunk order (0..7) gated on the
  DVE-staged uLo; pb streams derived rights with order (4..7,0..3) so
  its first links use the rights derived first.
- stores: ACT owns the pa half (copy+store, engine-local); DVE copies
  the pb half and the (post-fill idle) Sync queue stores it.
- frame 1024 (uHi of row 1023 only) is a 1-partition 8-matmul chain at
  the very end - its copies/stores are tiny, shortening the tail vs
  ending on a full tile.  Engine APs cannot start at partition 1, so it
  cannot be folded into a shifted-psum combine.
"""

import numpy as np
import ml_dtypes

import concourse.bass as bass
import concourse.bacc as bacc
import concourse.mybir as mybir
import concourse.tile as tile
from concourse import masks
from concourse.bass_utils import run_bass_kernel_spmd

B = 8
T = 1 << 20
R = 1024          # rows of x2 per channel (T // hop)
CN = 1024         # row width (hop) = N
NF = 1025         # output frames
NK = 1024         # output bins
F32 = mybir.dt.float32
BF16 = mybir.dt.bfloat16

_NC_CACHE = None
_CONST_CACHE = None


def build_nc() -> bass.Bass:
    nc = bacc.Bacc("TRN2", target_bir_lowering=False, debug=False)
    x = nc.dram_tensor("x", [R, CN], BF16, kind="ExternalInput").ap()
    w1r = nc.dram_tensor("w1r", [128, CN], BF16, kind="ExternalInput").ap()
    w2nr = nc.dram_tensor("w2nr", [128, CN], BF16, kind="ExternalInput").ap()
    d4l = nc.dram_tensor("d4l", [8, 128, 512], BF16, kind="ExternalInput").ap()
    sgr = nc.dram_tensor("sgr", [128, 128], BF16, kind="ExternalInput").ap()
    out = nc.dram_tensor("out", [NF, NK], BF16, kind="ExternalOutput").ap()

    xv = x.rearrange("(a p) c -> p a c", p=128)
    dv = d4l.rearrange("a p c -> p a c")

    with tile.TileContext(nc) as tc:
        with (
            tc.tile_pool(name="persist", bufs=1) as persist,
            tc.tile_pool(name="xin", bufs=1) as xin,
            tc.tile_pool(name="ypool", bufs=6) as ypool,
            tc.tile_pool(name="upool", bufs=4) as upool,
            tc.tile_pool(name="outp", bufs=4) as outp,
            tc.tile_pool(name="wps", bufs=1, space="PSUM") as wps,
            tc.tile_pool(name="tps", bufs=1, space="PSUM") as tps,
            tc.tile_pool(name="mmps", bufs=6, space="PSUM") as mmps,
        ):
            w1 = persist.tile([128, CN], BF16)
            w2n = persist.tile([128, CN], BF16)
            sgd = persist.tile([128, 128], BF16)

            ident = persist.tile([128, 128], BF16)
            masks.make_identity(nc, ident[:])
            nident = persist.tile([128, 128], BF16)
            nc.vector.tensor_scalar_mul(nident[:], ident[:], -1.0)

            dt = persist.tile([128, 8, NK], BF16)
            ulot = persist.tile([128, 4, R], BF16)
            uhit = persist.tile([128, 4, NF], BF16)
            nc.vector.memset(uhit[:, :, 0:1], 0.0)

            xts = [xin.tile([128, CN], BF16, name=f"xt{i}") for i in range(8)]

            # PE warmup: keep the PE continuously busy from the preamble
            # barrier so the HAM clock gate ramps to 2.4 GHz (~3.5 us of
            # sustained activity) before the derivation/DCT matmuls.
            warm = wps.tile([128, 128], BF16, tag="warm")
            for _ in range(38):
                nc.tensor.transpose(warm[:], ident[:], ident[:])

            # Fill DMAs (Sync queue), critical-path first.
            nc.sync.dma_start(sgd[:], sgr)
            nc.sync.dma_start(w1[:], w1r)
            nc.sync.dma_start(xts[0][:], xv[:, 0, :])
            nc.sync.dma_start(w2n[:], w2nr)
            nc.sync.dma_start(dt[:, 4:6, 0:512], dv[:, 4:6, :])
            nc.sync.dma_start(dt[:, 6:8, 0:512], dv[:, 6:8, :])
            nc.sync.dma_start(xts[1][:], xv[:, 1, :])
            nc.sync.dma_start(dt[:, 0:2, 0:512], dv[:, 0:2, :])
            nc.sync.dma_start(dt[:, 2:4, 0:512], dv[:, 2:4, :])
            for r in range(2, 8):
                nc.sync.dma_start(xts[r][:], xv[:, r, :])

            # Derive right half of D on the PE:
            #   dt[:, ci, 512+k] = s*sqrt2*dt[:, ci, k] - dt[:, ci, 511-k]
            def derive(ci):
                pd = mmps.tile([128, 512], F32, tag="mm")
                nc.tensor.matmul(pd[:], sgd[:], dt[:, ci, 0:512],
                                 start=True, stop=False)
                nc.tensor.matmul(pd[:], nident[:], dt[:, ci, 511::-1],
                                 start=False, stop=True)
                nc.scalar.copy(dt[:, ci, 512:1024], pd[:])

            def fold(r: int):
                xt = xts[r][:]
                r0 = r * 128
                pt = tps.tile([128, CN], BF16, tag="pt")
                y1 = ypool.tile([128, CN], BF16, tag="y1")
                un = upool.tile([128, CN], BF16)
                nc.vector.tensor_tensor(y1[:], xt, w1[:], mybir.AluOpType.mult)
                # uHi[p] = y1[p] - y1[1023-p]
                nc.vector.tensor_tensor(
                    un[:, 512:1024], y1[:, 0:512], y1[:, 1023:511:-1],
                    mybir.AluOpType.subtract,
                )
                for ci in range(4):
                    nc.tensor.transpose(
                        pt[:, ci * 128:(ci + 1) * 128],
                        un[:, 512 + ci * 128:512 + (ci + 1) * 128], ident[:],
                    )
                nc.scalar.copy(uhit[:, 0:4, 1 + r0:1 + r0 + 128], pt[:, 0:512])
                y2n = ypool.tile([128, CN], BF16, tag="y2n")
                nc.vector.tensor_tensor(y2n[:], xt, w2n[:], mybir.AluOpType.mult)
                # uLo[m] = y2n[511-m] + y2n[512+m]   (y2n = -w2*x)
                nc.vector.tensor_tensor(
                    un[:, 0:512], y2n[:, 511::-1], y2n[:, 512:1024],
                    mybir.AluOpType.add,
                )
                for ci in range(4):
                    nc.tensor.transpose(
                        pt[:, 512 + ci * 128:512 + (ci + 1) * 128],
                        un[:, ci * 128:(ci + 1) * 128], ident[:],
                    )
                nc.vector.tensor_copy(ulot[:, 0:4, r0:r0 + 128], pt[:, 512:1024])

            def wslice(ci, f0):
                if ci < 4:
                    return ulot[:, ci, f0:f0 + 128]
                return uhit[:, ci - 4, f0:f0 + 128]

            CHAIN_A = (0, 1, 2, 3, 4, 5, 6, 7)
            CHAIN_B = (4, 5, 6, 7, 0, 1, 2, 3)

            def dct_tile(j: int):
                f0 = j * 128
                ot = outp.tile([128, NK], BF16)
                pa = mmps.tile([128, 512], F32, tag="mm")
                for ci in CHAIN_A:
                    nc.tensor.matmul(
                        pa[:], wslice(ci, f0), dt[:, ci, 0:512],
                        start=(ci == CHAIN_A[0]), stop=(ci == CHAIN_A[-1]),
                    )
                nc.scalar.copy(ot[:, 0:512], pa[:])
                nc.scalar.dma_start(out[f0:f0 + 128, 0:512], ot[:, 0:512])
                pb = mmps.tile([128, 512], F32, tag="mm")
                for ci in CHAIN_B:
                    nc.tensor.matmul(
                        pb[:], wslice(ci, f0), dt[:, ci, 512:1024],
                        start=(ci == CHAIN_B[0]), stop=(ci == CHAIN_B[-1]),
                    )
                nc.vector.tensor_copy(ot[:, 512:1024], pb[:])
                nc.sync.dma_start(out[f0:f0 + 128, 512:1024], ot[:, 512:1024])

            def last_frame():
                # f=1024: only the uHi half (row 1023) contributes.
                pa = mmps.tile([1, 512], F32, tag="mm")
                pb = mmps.tile([1, 512], F32, tag="mm")
                for ci in range(4):
                    wsl = uhit[:, ci, 1024:1025]
                    nc.tensor.matmul(
                        pa[:], wsl, dt[:, 4 + ci, 0:512],
                        start=(ci == 0), stop=(ci == 3),
                    )
                    nc.tensor.matmul(
                        pb[:], wsl, dt[:, 4 + ci, 512:1024],
                        start=(ci == 0), stop=(ci == 3),
                    )
                ot = outp.tile([1, NK], BF16, tag="ot_last")
                nc.scalar.copy(ot[:, 0:512], pa[:])
                nc.scalar.dma_start(out[1024:1025, 0:512], ot[:, 0:512])
                nc.vector.tensor_copy(ot[:, 512:1024], pb[:])
                nc.sync.dma_start(out[1024:1025, 512:1024], ot[:, 512:1024])

            for ci in (4, 5, 6, 7):
                derive(ci)
            fold(0)
            for ci in (0, 1, 2, 3):
                derive(ci)
            dct_tile(0)
            for r in range(1, 8):
                fold(r)
                dct_tile(r)
            last_frame()

    return nc


def make_consts(window: np.ndarray):
    w = window.astype(np.float64)
    w1r = np.broadcast_to(w[:CN].astype(ml_dtypes.bfloat16), (128, CN)).copy()
    w2nr = np.broadcast_to((-w[CN:]).astype(ml_dtypes.bfloat16), (128, CN)).copy()
    m = np.arange(NK, dtype=np.float64)[:, None]
    k = np.arange(NK, dtype=np.float64)[None, :]
    d = (np.sqrt(2.0 / NK) * np.cos(np.pi / NK * (m + 0.5) * (k + 0.5)))
    d4l = np.ascontiguousarray(
        d.astype(ml_dtypes.bfloat16).reshape(8, 128, NK)[:, :, :512])
    p = np.arange(128)
    s = np.where(np.isin(p % 4, [0, 3]), np.sqrt(2.0), -np.sqrt(2.0))
    sgr = np.diag(s).astype(ml_dtypes.bfloat16)
    return w1r, w2nr, d4l, sgr


def _get_nc() -> bass.Bass:
    global _NC_CACHE
    if _NC_CACHE is None:
        _NC_CACHE = build_nc()
        _NC_CACHE.compile()
    return _NC_CACHE


def run_spmd(x: np.ndarray, window: np.ndarray, **kwargs):
    """Shard, run on 8 cores, return (stacked output, BassKernelResults)."""
    global _CONST_CACHE
    if _CONST_CACHE is None or _CONST_CACHE[0] != window.tobytes():
        _CONST_CACHE = (window.tobytes(), make_consts(window))
    w1r, w2nr, d4l, sgr = _CONST_CACHE[1]
    in_maps = [
        {"x": np.ascontiguousarray(
            x[b].reshape(R, CN).astype(ml_dtypes.bfloat16)),
         "w1r": w1r, "w2nr": w2nr, "d4l": d4l, "sgr": sgr}
        for b in range(B)
    ]
    res = run_bass_kernel_spmd(nc=_get_nc(), in_maps=in_maps,
                               core_ids=list(range(B)), **kwargs)
    out = np.stack([res.results[b]["out"].astype(np.float32) for b in range(B)],
                   axis=0)
    return out, res


def kernel(x: np.ndarray, window: np.ndarray) -> np.ndarray:
    out, _ = run_spmd(np.asarray(x), np.asarray(window))
    return out


# revision 12
# speedup vs baseline: 1.1284x; 1.1284x over previous
"""MDCT kernel for Trainium2 (8 NeuronCores, batch-parallel), folded DCT-IV form.

Math: frame f (hop N=1024, len 2N, center-padded) folds to an N-vector u and
out[f] = DCT-IV(u).  With x2 = x.reshape(1024, 1024) and y1 = w[:N]*x2[r],
y2 = w[N:]*x2[r] (per-row windowing):
    u[f, m]      = -y2[f, 511-m] - y2[f, 512+m]      (m < 512,  row f)
    u[f, 512+p]  =  y1[f-1, p]   - y1[f-1, 1023-p]   (p < 512,  row f-1)
so each x2 row r yields uLo[r] (frame r) and uHi[r] (frame r+1), and
    out[f, k] = sum_m u[f, m] * D4[m, k],   D4 = sqrt(2/N) DCT-IV matrix.

This halves the matmul contraction (1024 vs 2048) vs the direct form.  The
fold runs on the vector engine (reversals are negative-stride APs), u is
transposed on the PE in bf16 (1 cyc/row), and the DCT matmuls run in bf16.

Only the LEFT half of D4 is DMA'd (1.05 MB instead of 2.1 MB): the DCT-IV
matrix satisfies  D4[m, 512+k] = s_m*sqrt(2)*D4[m, k] - D4[m, 511-k]  with
s_m = +1 for m%4 in {0,3} else -1.  The right half is derived ON THE PE
(idle during the fill) as a 2-matmul PSUM chain per 128-row chunk:
diag(s*sqrt2) @ Dl  accumulated with  (-I) @ Dl-reversed-columns, then an
ACT copy back to SBUF.  (GPSIMD tensor_scalar takes 7.6us per [128,512]
chunk - far too slow; DVE has no spare capacity early.)

Schedule notes (v5, from NTFF profiles of baseline=53.8us, v2=66.8us,
v3=124us, v4=57.6us):
- the NEFF preamble (engine iram loads, const memsets, a global barrier)
  ends ~7.2 us; nothing (including DMA) starts before that.
- concurrent DMA streams share ~400 GB/s; completion of any transfer is
  set by cumulative bytes ahead of it, so the DCT gate is the byte count
  of w + x0 + Dl (~1.8 MB) rather than instruction order tricks.
- the HAM clock gate resets on any PE idle gap: warmup transposes must
  keep the PE continuously busy from the preamble barrier until real
  work arrives, or the DCT stream starts at 1.2 GHz (v4 lost ~2 us).
- per-tile chains: pa streams Dl with chunk order (0..7) gated on the
  DVE-staged uLo; pb streams derived rights with order (4..7,0..3) so
  its first links use the rights derived first.
- stores: ACT owns the pa half (copy+store, engine-local); DVE copies
  the pb half and the (post-fill idle) Sync queue stores it.
- frame 1024 (uHi of row 1023 only) is a 1-partition 8-matmul chain at
  the very end - its copies/stores are tiny, shortening the tail vs
  ending on a full tile.  Engine APs cannot start at partition 1, so it
  cannot be folded into a shifted-psum combine.
"""

import numpy as np
import ml_dtypes

import concourse.bass as bass
import concourse.bacc as bacc
import concourse.mybir as mybir
import concourse.tile as tile
from concourse import masks
from concourse.bass_utils import run_bass_kernel_spmd

B = 8
T = 1 << 20
R = 1024          # rows of x2 per channel (T // hop)
CN = 1024         # row width (hop) = N
NF = 1025         # output frames
NK = 1024         # output bins
F32 = mybir.dt.float32
BF16 = mybir.dt.bfloat16

_NC_CACHE = None
_CONST_CACHE = None


def build_nc() -> bass.Bass:
    nc = bacc.Bacc("TRN2", target_bir_lowering=False, debug=False)
    x = nc.dram_tensor("x", [R, CN], BF16, kind="ExternalInput").ap()
    w1r = nc.dram_tensor("w1r", [128, CN], BF16, kind="ExternalInput").ap()
    w2nr = nc.dram_tensor("w2nr", [128, CN], BF16, kind="ExternalInput").ap()
    d4l = nc.dram_tensor("d4l", [8, 128, 512], BF16, kind="ExternalInput").ap()
    sgr = nc.dram_tensor("sgr", [128, 128], BF16, kind="ExternalInput").ap()
    out = nc.dram_tensor("out", [NF, NK], BF16, kind="ExternalOutput").ap()

    xv = x.rearrange("(a p) c -> p a c", p=128)
    dv = d4l.rearrange("a p c -> p a c")

    with tile.TileContext(nc) as tc:
        with (
            tc.tile_pool(name="persist", bufs=1) as persist,
            tc.tile_pool(name="xin", bufs=1) as xin,
            tc.tile_pool(name="ypool", bufs=6) as ypool,
            tc.tile_pool(name="upool", bufs=4) as upool,
            tc.tile_pool(name="outp", bufs=4) as outp,
            tc.tile_pool(name="wps", bufs=1, space="PSUM") as wps,
            tc.tile_pool(name="tps", bufs=2, space="PSUM") as tps,
            tc.tile_pool(name="mmps", bufs=5, space="PSUM") as mmps,
        ):
            w1 = persist.tile([128, CN], BF16)
            w2n = persist.tile([128, CN], BF16)
            sgd = persist.tile([128, 128], BF16)

            ident = persist.tile([128, 128], BF16)
            masks.make_identity(nc, ident[:])
            nident = persist.tile([128, 128], BF16)
            nc.vector.tensor_scalar_mul(nident[:], ident[:], -1.0)

            dt = persist.tile([128, 8, NK], BF16)
            ulot = persist.tile([128, 4, R], BF16)
            uhit = persist.tile([128, 4, NF], BF16)
            nc.vector.memset(uhit[:, :, 0:1], 0.0)

            xts = [xin.tile([128, CN], BF16, name=f"xt{i}") for i in range(8)]

            # PE warmup: keep the PE continuously busy from the preamble
            # barrier so the HAM clock gate ramps to 2.4 GHz (~3.5 us of
            # sustained activity) before the derivation/DCT matmuls.
            warm = wps.tile([128, 128], BF16, tag="warm")
            for _ in range(50):
                nc.tensor.transpose(warm[:], ident[:], ident[:])

            # Fill DMAs (Sync queue), critical-path first.
            nc.sync.dma_start(sgd[:], sgr)
            nc.sync.dma_start(w1[:], w1r)
            nc.sync.dma_start(xts[0][:], xv[:, 0, :])
            nc.sync.dma_start(w2n[:], w2nr)
            nc.sync.dma_start(dt[:, 4:6, 0:512], dv[:, 4:6, :])
            nc.sync.dma_start(dt[:, 6:8, 0:512], dv[:, 6:8, :])
            nc.sync.dma_start(xts[1][:], xv[:, 1, :])
            nc.sync.dma_start(dt[:, 0:2, 0:512], dv[:, 0:2, :])
            nc.sync.dma_start(dt[:, 2:4, 0:512], dv[:, 2:4, :])
            for r in range(2, 8):
                nc.sync.dma_start(xts[r][:], xv[:, r, :])

            # Derive right half of D on the PE:
            #   dt[:, ci, 512+k] = s*sqrt2*dt[:, ci, k] - dt[:, ci, 511-k]
            def derive(ci):
                pd = mmps.tile([128, 512], F32, tag="mm")
                nc.tensor.matmul(pd[:], sgd[:], dt[:, ci, 0:512],
                                 start=True, stop=False)
                nc.tensor.matmul(pd[:], nident[:], dt[:, ci, 511::-1],
                                 start=False, stop=True)
                nc.scalar.copy(dt[:, ci, 512:1024], pd[:])

            def fold(r: int):
                xt = xts[r][:]
                r0 = r * 128
                pt = tps.tile([128, CN], BF16, tag="pt")
                y1 = ypool.tile([128, CN], BF16, tag="y1")
                un = upool.tile([128, CN], BF16)
                nc.vector.tensor_tensor(y1[:], xt, w1[:], mybir.AluOpType.mult)
                # uHi[p] = y1[p] - y1[1023-p]
                nc.vector.tensor_tensor(
                    un[:, 512:1024], y1[:, 0:512], y1[:, 1023:511:-1],
                    mybir.AluOpType.subtract,
                )
                for ci in range(4):
                    nc.tensor.transpose(
                        pt[:, ci * 128:(ci + 1) * 128],
                        un[:, 512 + ci * 128:512 + (ci + 1) * 128], ident[:],
                    )
                nc.scalar.copy(uhit[:, 0:4, 1 + r0:1 + r0 + 128], pt[:, 0:512])
                y2n = ypool.tile([128, CN], BF16, tag="y2n")
                nc.vector.tensor_tensor(y2n[:], xt, w2n[:], mybir.AluOpType.mult)
                # uLo[m] = y2n[511-m] + y2n[512+m]   (y2n = -w2*x)
                nc.vector.tensor_tensor(
                    un[:, 0:512], y2n[:, 511::-1], y2n[:, 512:1024],
                    mybir.AluOpType.add,
                )
                for ci in range(4):
                    nc.tensor.transpose(
                        pt[:, 512 + ci * 128:512 + (ci + 1) * 128],
                        un[:, ci * 128:(ci + 1) * 128], ident[:],
                    )
                nc.vector.tensor_copy(ulot[:, 0:4, r0:r0 + 128], pt[:, 512:1024])

            def wslice(ci, f0):
                if ci < 4:
                    return ulot[:, ci, f0:f0 + 128]
                return uhit[:, ci - 4, f0:f0 + 128]

            CHAIN_A = (0, 1, 2, 3, 4, 5, 6, 7)
            CHAIN_B = (4, 5, 6, 7, 0, 1, 2, 3)

            def dct_tile(j: int):
                f0 = j * 128
                ot = outp.tile([128, NK], BF16)
                pa = mmps.tile([128, 512], F32, tag="mm")
                for ci in CHAIN_A:
                    nc.tensor.matmul(
                        pa[:], wslice(ci, f0), dt[:, ci, 0:512],
                        start=(ci == CHAIN_A[0]), stop=(ci == CHAIN_A[-1]),
                    )
                nc.scalar.copy(ot[:, 0:512], pa[:])
                nc.scalar.dma_start(out[f0:f0 + 128, 0:512], ot[:, 0:512])
                pb = mmps.tile([128, 512], F32, tag="mm")
                for ci in CHAIN_B:
                    nc.tensor.matmul(
                        pb[:], wslice(ci, f0), dt[:, ci, 512:1024],
                        start=(ci == CHAIN_B[0]), stop=(ci == CHAIN_B[-1]),
                    )
                nc.vector.tensor_copy(ot[:, 512:1024], pb[:])
                nc.sync.dma_start(out[f0:f0 + 128, 512:1024], ot[:, 512:1024])

            def last_frame():
                # f=1024: only the uHi half (row 1023) contributes.
                pa = mmps.tile([1, 512], F32, tag="mm")
                pb = mmps.tile([1, 512], F32, tag="mm")
                for ci in range(4):
                    wsl = uhit[:, ci, 1024:1025]
                    nc.tensor.matmul(
                        pa[:], wsl, dt[:, 4 + ci, 0:512],
                        start=(ci == 0), stop=(ci == 3),
                    )
                    nc.tensor.matmul(
                        pb[:], wsl, dt[:, 4 + ci, 512:1024],
                        start=(ci == 0), stop=(ci == 3),
                    )
                ot = outp.tile([1, NK], BF16, tag="ot_last")
                nc.scalar.copy(ot[:, 0:512], pa[:])
                nc.scalar.dma_start(out[1024:1025, 0:512], ot[:, 0:512])
                nc.vector.tensor_copy(ot[:, 512:1024], pb[:])
                nc.sync.dma_start(out[1024:1025, 512:1024], ot[:, 512:1024])

            for ci in (4, 5, 6, 7):
                derive(ci)
            fold(0)
            for ci in (0, 1, 2, 3):
                derive(ci)
            dct_tile(0)
            for r in range(1, 8):
                fold(r)
                dct_tile(r)
            last_frame()

    return nc


def make_consts(window: np.ndarray):
    w = window.astype(np.float64)
    w1r = np.broadcast_to(w[:CN].astype(ml_dtypes.bfloat16), (128, CN)).copy()
    w2nr = np.broadcast_to((-w[CN:]).astype(ml_dtypes.bfloat16), (128, CN)).copy()
    m = np.arange(NK, dtype=np.float64)[:, None]
    k = np.arange(NK, dtype=np.float64)[None, :]
    d = (np.sqrt(2.0 / NK) * np.cos(np.pi / NK * (m + 0.5) * (k + 0.5)))
    d4l = np.ascontiguousarray(
        d.astype(ml_dtypes.bfloat16).reshape(8, 128, NK)[:, :, :512])
    p = np.arange(128)
    s = np.where(np.isin(p % 4, [0, 3]), np.sqrt(2.0), -np.sqrt(2.0))
    sgr = np.diag(s).astype(ml_dtypes.bfloat16)
    return w1r, w2nr, d4l, sgr


def _get_nc() -> bass.Bass:
    global _NC_CACHE
    if _NC_CACHE is None:
        _NC_CACHE = build_nc()
        _NC_CACHE.compile()
    return _NC_CACHE


def run_spmd(x: np.ndarray, window: np.ndarray, **kwargs):
    """Shard, run on 8 cores, return (stacked output, BassKernelResults)."""
    global _CONST_CACHE
    if _CONST_CACHE is None or _CONST_CACHE[0] != window.tobytes():
        _CONST_CACHE = (window.tobytes(), make_consts(window))
    w1r, w2nr, d4l, sgr = _CONST_CACHE[1]
    in_maps = [
        {"x": np.ascontiguousarray(
            x[b].reshape(R, CN).astype(ml_dtypes.bfloat16)),
         "w1r": w1r, "w2nr": w2nr, "d4l": d4l, "sgr": sgr}
        for b in range(B)
    ]
    res = run_bass_kernel_spmd(nc=_get_nc(), in_maps=in_maps,
                               core_ids=list(range(B)), **kwargs)
    out = np.stack([res.results[b]["out"].astype(np.float32) for b in range(B)],
                   axis=0)
    return out, res


def kernel(x: np.ndarray, window: np.ndarray) -> np.ndarray:
    out, _ = run_spmd(np.asarray(x), np.asarray(window))
    return out


# revision 15
# speedup vs baseline: 1.2026x; 1.0658x over previous
"""MDCT kernel for Trainium2 (8 NeuronCores, batch-parallel), folded DCT-IV form.

Math: frame f (hop N=1024, len 2N, center-padded) folds to an N-vector u and
out[f] = DCT-IV(u).  With x2 = x.reshape(1024, 1024) and y1 = w[:N]*x2[r],
y2 = w[N:]*x2[r] (per-row windowing):
    u[f, m]      = -y2[f, 511-m] - y2[f, 512+m]      (m < 512,  row f)
    u[f, 512+p]  =  y1[f-1, p]   - y1[f-1, 1023-p]   (p < 512,  row f-1)
so each x2 row r yields uLo[r] (frame r) and uHi[r] (frame r+1), and
    out[f, k] = sum_m u[f, m] * D4[m, k],   D4 = sqrt(2/N) DCT-IV matrix.

Only the LEFT half of D4 is shipped (1.05 MB instead of 2.1 MB), using
    D4[m, 512+k] = s_m*sqrt(2)*D4[m, k] - D4[m, 511-k],
    s_m = +1 for m%4 in {0,3} else -1,
which for the output means
    out[f, 512+k] = pa'[f, k] - pa[f, 511-k]
where pa = u^T Dl (the left-half chain) and pa' is the SAME chain with
sign-scaled weights u' = sqrt(2)*s*u.  u' costs nothing on the PE: the
scale is applied by the PSUM->SBUF staging copies (per-partition scale AP
on ACT, tensor_scalar on DVE), and the final combine (pa' minus
column-reversed pa) replaces the plain pb copy at identical DVE cost.
So PE work is identical to the full-D kernel while the DCT-gating fill
drops by 1.05 MB (~2.7 us at the ~400 GB/s shared-DMA rate).

Schedule notes (v7; NTFF profiles: baseline=53.8us, batched-DMA=66.8us,
gpsimd-derive=124us, v4=57.6us, PE-derive=69.7/61.8us):
- the NEFF preamble ends ~7.2 us; nothing (not even DMA) starts earlier.
- concurrent DMA streams share ~400 GB/s; a transfer completes when the
  cumulative bytes ahead of it have streamed, so the DCT gate is the
  byte count of x0+w+x1+Dl (~2.05 MB), cleared ~12.8 us.
- warmup transposes keep the PE continuously busy from the preamble to
  the first fold transposes: any >3.4 us PE idle re-throttles the HAM
  clock to 1.2 GHz and the ramp back takes ~3.5 us of sustained work.
- proven baseline software pipeline: fold(r+2) is emitted before
  dct_tile(r) so fold transposes interleave into chain-link stalls.
- engine ownership: DVE = folds + uLo staging + the pa'-pa combine;
  ACT = uHi/uHi'/uLo' staging + pa copy + a-half stores (engine-local);
  Sync = fills + b-half stores.  GPSIMD is unusable (no PSUM access,
  ~7.6us per [128,512] tensor_scalar).
- frame 1024 (uHi of row 1023 only) runs as a 1-partition chain at the
  very end; its copies/stores are tiny so the drain tail is short.
  Engine APs cannot start at partition 1, so it cannot become a
  shifted-psum combine.
"""

import numpy as np
import ml_dtypes

import concourse.bass as bass
import concourse.bacc as bacc
import concourse.mybir as mybir
import concourse.tile as tile
from concourse import masks
from concourse.bass_utils import run_bass_kernel_spmd

B = 8
T = 1 << 20
R = 1024          # rows of x2 per channel (T // hop)
CN = 1024         # row width (hop) = N
NF = 1025         # output frames
NK = 1024         # output bins
F32 = mybir.dt.float32
BF16 = mybir.dt.bfloat16

_NC_CACHE = None
_CONST_CACHE = None


def build_nc() -> bass.Bass:
    nc = bacc.Bacc("TRN2", target_bir_lowering=False, debug=False)
    x = nc.dram_tensor("x", [R, CN], BF16, kind="ExternalInput").ap()
    w1r = nc.dram_tensor("w1r", [128, CN], BF16, kind="ExternalInput").ap()
    w2nr = nc.dram_tensor("w2nr", [128, CN], BF16, kind="ExternalInput").ap()
    d4l = nc.dram_tensor("d4l", [8, 128, 512], BF16, kind="ExternalInput").ap()
    svr = nc.dram_tensor("svr", [128, 1], F32, kind="ExternalInput").ap()
    out = nc.dram_tensor("out", [NF, NK], BF16, kind="ExternalOutput").ap()

    xv = x.rearrange("(a p) c -> p a c", p=128)
    dv = d4l.rearrange("a p c -> p a c")

    with tile.TileContext(nc) as tc:
        with (
            tc.tile_pool(name="persist", bufs=1) as persist,
            tc.tile_pool(name="xin", bufs=1) as xin,
            tc.tile_pool(name="ypool", bufs=6) as ypool,
            tc.tile_pool(name="upool", bufs=4) as upool,
            tc.tile_pool(name="outp", bufs=4) as outp,
            tc.tile_pool(name="wps", bufs=1, space="PSUM") as wps,
            tc.tile_pool(name="tps", bufs=2, space="PSUM") as tps,
            tc.tile_pool(name="mmps", bufs=4, space="PSUM") as mmps,
        ):
            w1 = persist.tile([128, CN], BF16)
            w2n = persist.tile([128, CN], BF16)
            sv = persist.tile([128, 1], F32)

            ident = persist.tile([128, 128], BF16)
            masks.make_identity(nc, ident[:])

            dt = persist.tile([128, 8, 512], BF16)
            ulot = persist.tile([128, 4, R], BF16)
            uhit = persist.tile([128, 4, NF], BF16)
            ulotp = persist.tile([128, 4, R], BF16)
            uhitp = persist.tile([128, 4, NF], BF16)
            nc.vector.memset(uhit[:, :, 0:1], 0.0)
            nc.vector.memset(uhitp[:, :, 0:1], 0.0)

            xts = [xin.tile([128, CN], BF16, name=f"xt{i}") for i in range(8)]

            # PE warmup: keep the PE continuously busy from the preamble
            # barrier until fold(0)'s transposes, so the HAM clock gate
            # ramps to 2.4 GHz before the DCT stream starts.
            warm = wps.tile([128, 128], BF16, tag="warm")
            for _ in range(55):
                nc.tensor.transpose(warm[:], ident[:], ident[:])

            # Fill DMAs (Sync queue), DCT-gating bytes first.
            nc.sync.dma_start(sv[:], svr)
            nc.sync.dma_start(xts[0][:], xv[:, 0, :])
            nc.sync.dma_start(w1[:], w1r)
            nc.sync.dma_start(w2n[:], w2nr)
            nc.sync.dma_start(xts[1][:], xv[:, 1, :])
            nc.sync.dma_start(dt[:, 0:2, :], dv[:, 0:2, :])
            nc.sync.dma_start(dt[:, 2:4, :], dv[:, 2:4, :])
            nc.sync.dma_start(dt[:, 4:6, :], dv[:, 4:6, :])
            nc.sync.dma_start(dt[:, 6:8, :], dv[:, 6:8, :])
            for r in range(2, 8):
                nc.sync.dma_start(xts[r][:], xv[:, r, :])

            def fold(r: int):
                xt = xts[r][:]
                r0 = r * 128
                pt = tps.tile([128, CN], BF16, tag="pt")
                y1 = ypool.tile([128, CN], BF16, tag="y1")
                un = upool.tile([128, CN], BF16)
                nc.vector.tensor_tensor(y1[:], xt, w1[:], mybir.AluOpType.mult)
                # uHi[p] = y1[p] - y1[1023-p]
                nc.vector.tensor_tensor(
                    un[:, 512:1024], y1[:, 0:512], y1[:, 1023:511:-1],
                    mybir.AluOpType.subtract,
                )
                for ci in range(4):
                    nc.tensor.transpose(
                        pt[:, ci * 128:(ci + 1) * 128],
                        un[:, 512 + ci * 128:512 + (ci + 1) * 128], ident[:],
                    )
                nc.scalar.copy(uhit[:, 0:4, 1 + r0:1 + r0 + 128], pt[:, 0:512])
                nc.scalar.mul(uhitp[:, 0:4, 1 + r0:1 + r0 + 128], pt[:, 0:512],
                              sv[:, 0:1])
                y2n = ypool.tile([128, CN], BF16, tag="y2n")
                nc.vector.tensor_tensor(y2n[:], xt, w2n[:], mybir.AluOpType.mult)
                # uLo[m] = y2n[511-m] + y2n[512+m]   (y2n = -w2*x)
                nc.vector.tensor_tensor(
                    un[:, 0:512], y2n[:, 511::-1], y2n[:, 512:1024],
                    mybir.AluOpType.add,
                )
                for ci in range(4):
                    nc.tensor.transpose(
                        pt[:, 512 + ci * 128:512 + (ci + 1) * 128],
                        un[:, ci * 128:(ci + 1) * 128], ident[:],
                    )
                nc.vector.tensor_copy(ulot[:, 0:4, r0:r0 + 128], pt[:, 512:1024])
                nc.scalar.mul(ulotp[:, 0:4, r0:r0 + 128], pt[:, 512:1024],
                              sv[:, 0:1])

            def wslice(ci, f0, primed):
                lo, hi = (ulotp, uhitp) if primed else (ulot, uhit)
                if ci < 4:
                    return lo[:, ci, f0:f0 + 128]
                return hi[:, ci - 4, f0:f0 + 128]

            def dct_tile(j: int):
                f0 = j * 128
                ot = outp.tile([128, NK], BF16)
                pa = mmps.tile([128, 512], F32, tag="mm")
                for ci in range(8):
                    nc.tensor.matmul(
                        pa[:], wslice(ci, f0, False), dt[:, ci, :],
                        start=(ci == 0), stop=(ci == 7),
                    )
                nc.scalar.copy(ot[:, 0:512], pa[:])
                nc.scalar.dma_start(out[f0:f0 + 128, 0:512], ot[:, 0:512])
                pp = mmps.tile([128, 512], F32, tag="mm")
                for ci in range(8):
                    nc.tensor.matmul(
                        pp[:], wslice(ci, f0, True), dt[:, ci, :],
                        start=(ci == 0), stop=(ci == 7),
                    )
                # out[:, 512+k] = pa'[k] - pa[511-k]; the reversed pa
                # operand reads the SBUF copy (ot a-half) because engines
                # cannot read two PSUM inputs in one instruction.
                nc.vector.tensor_tensor(
                    ot[:, 512:1024], pp[:], ot[:, 511::-1],
                    mybir.AluOpType.subtract,
                )
                nc.sync.dma_start(out[f0:f0 + 128, 512:1024], ot[:, 512:1024])

            def last_frame():
                # f=1024: only the uHi half (row 1023) contributes.
                pa = mmps.tile([1, 512], F32, tag="mm")
                pp = mmps.tile([1, 512], F32, tag="mm")
                for ci in range(4):
                    nc.tensor.matmul(
                        pa[:], uhit[:, ci, 1024:1025], dt[:, 4 + ci, :],
                        start=(ci == 0), stop=(ci == 3),
                    )
                    nc.tensor.matmul(
                        pp[:], uhitp[:, ci, 1024:1025], dt[:, 4 + ci, :],
                        start=(ci == 0), stop=(ci == 3),
                    )
                ot = outp.tile([1, NK], BF16, tag="ot_last")
                nc.scalar.copy(ot[:, 0:512], pa[:])
                nc.scalar.dma_start(out[1024:1025, 0:512], ot[:, 0:512])
                nc.vector.tensor_tensor(
                    ot[:, 512:1024], pp[:], ot[:, 511::-1],
                    mybir.AluOpType.subtract,
                )
                nc.sync.dma_start(out[1024:1025, 512:1024], ot[:, 512:1024])

            fold(0)
            fold(1)
            for r in range(8):
                if r + 2 < 8:
                    fold(r + 2)
                dct_tile(r)
            last_frame()

    return nc


def make_consts(window: np.ndarray):
    w = window.astype(np.float64)
    w1r = np.broadcast_to(w[:CN].astype(ml_dtypes.bfloat16), (128, CN)).copy()
    w2nr = np.broadcast_to((-w[CN:]).astype(ml_dtypes.bfloat16), (128, CN)).copy()
    m = np.arange(NK, dtype=np.float64)[:, None]
    k = np.arange(NK, dtype=np.float64)[None, :]
    d = (np.sqrt(2.0 / NK) * np.cos(np.pi / NK * (m + 0.5) * (k + 0.5)))
    d4l = np.ascontiguousarray(
        d.astype(ml_dtypes.bfloat16).reshape(8, 128, NK)[:, :, :512])
    p = np.arange(128)
    svr = np.where(np.isin(p % 4, [0, 3]), np.sqrt(2.0), -np.sqrt(2.0))
    svr = svr.reshape(128, 1).astype(np.float32)
    return w1r, w2nr, d4l, svr


def _get_nc() -> bass.Bass:
    global _NC_CACHE
    if _NC_CACHE is None:
        _NC_CACHE = build_nc()
        _NC_CACHE.compile()
    return _NC_CACHE


def run_spmd(x: np.ndarray, window: np.ndarray, **kwargs):
    """Shard, run on 8 cores, return (stacked output, BassKernelResults)."""
    global _CONST_CACHE
    if _CONST_CACHE is None or _CONST_CACHE[0] != window.tobytes():
        _CONST_CACHE = (window.tobytes(), make_consts(window))
    w1r, w2nr, d4l, svr = _CONST_CACHE[1]
    in_maps = [
        {"x": np.ascontiguousarray(
            x[b].reshape(R, CN).astype(ml_dtypes.bfloat16)),
         "w1r": w1r, "w2nr": w2nr, "d4l": d4l, "svr": svr}
        for b in range(B)
    ]
    res = run_bass_kernel_spmd(nc=_get_nc(), in_maps=in_maps,
                               core_ids=list(range(B)), **kwargs)
    out = np.stack([res.results[b]["out"].astype(np.float32) for b in range(B)],
                   axis=0)
    return out, res


def kernel(x: np.ndarray, window: np.ndarray) -> np.ndarray:
    out, _ = run_spmd(np.asarray(x), np.asarray(window))
    return out
